# revision 1
# baseline (speedup 1.0000x reference)
"""Trainium2 Bass kernel for GQA attention (32 q heads / 16 kv heads, head_dim
128, L=2048, D=4608) with RoPE, tanh softcap 50, causal mask, o_proj.

Strategy: tensor-parallel over heads across 8 NeuronCores. Core c computes
q-heads 4c..4c+3 and kv-heads 2c..2c+1 end-to-end (QKV projections, RoPE,
softcapped causal attention, and the partial o_proj against its 512 columns of
wo); the host sums the 8 partial [L, D] outputs.

Per-core layout choices:
  - host passes x, weights pre-transposed (xT [D,L], wqT/wkT/wvT [D,*],
    woT [512,D]) and bf16-cast; all matmuls run bf16 with fp32 PSUM accumulation
  - projections produce QT/KT/VT in [head_dim, L] layout directly
  - RoPE applied in [d, l] layout via host cos/sin tables ([128, L]; sin table
    carries the rotate-half sign split); the 1/sqrt(144) q-scale is folded into
    the softcap activation scale
  - scores computed transposed, ST[k, q] = (KT tile)^T @ QT chunk, so the
    exp(softcap) output PT[k, q] feeds the PV matmul as lhsT with no transpose
  - softcap bounds scores to +-50 so softmax needs no max-subtraction:
    p = exp(50*tanh(s/600)), denominator = sum_k p obtained free via a
    ones-column appended to the V rhs tile
  - attention output [q, d] is divided by the denominator, PE-transposed to
    [d, q], and accumulated into the o_proj over the 4 local heads
"""

import os

import numpy as np
import ml_dtypes

import concourse.bass as bass
import concourse.mybir as mybir
import concourse.tile as tile
from concourse.masks import make_identity
from concourse import bacc

F32 = mybir.dt.float32
BF16 = mybir.dt.bfloat16
BF16_NP = ml_dtypes.bfloat16
AF = mybir.ActivationFunctionType

N_HEADS = 32
N_KV = 16
HEAD_DIM = 128
ROPE_THETA = 10000.0
SOFTCAP = 50.0
SCALE = 1.0 / 12.0  # 1/sqrt(144)
L = 2048
D = 4608
N_CORES = 8
QH = N_HEADS // N_CORES        # 4 local q heads
KVH = N_KV // N_CORES          # 2 local kv heads
KC = D // 128                  # 36 contraction chunks
NQ = L // 512                  # 4 l-chunks of 512
LT = L // 128                  # 16 l-tiles of 128
DOUT_CHUNKS = D // 512         # 9 o_proj output chunks


def _emit(nc, n_iters=1):
    xt_d = nc.dram_tensor("xt", [D, L], BF16, kind="ExternalInput")
    wqt_d = nc.dram_tensor("wqt", [D, QH * 128], BF16, kind="ExternalInput")
    wkt_d = nc.dram_tensor("wkt", [D, KVH * 128], BF16, kind="ExternalInput")
    wvt_d = nc.dram_tensor("wvt", [D, KVH * 128], BF16, kind="ExternalInput")
    wot_d = nc.dram_tensor("wot", [QH * 128, D], BF16, kind="ExternalInput")
    cost_d = nc.dram_tensor("cost", [128, L], BF16, kind="ExternalInput")
    sint_d = nc.dram_tensor("sint", [128, L], BF16, kind="ExternalInput")
    masks_d = nc.dram_tensor("masks", [4, 128, 512], BF16, kind="ExternalInput")
    out_d = nc.dram_tensor("out", [L, D], F32, kind="ExternalOutput")

    with tile.TileContext(nc) as tc:
        with (
            tc.tile_pool(name="const", bufs=1) as const,
            tc.tile_pool(name="persist", bufs=1) as persist,
        ):
            ident = const.tile([128, 128], BF16)
            make_identity(nc, ident[:])
            cost = const.tile([128, L], BF16)
            sint = const.tile([128, L], BF16)
            nc.sync.dma_start(cost[:], cost_d[:])
            nc.sync.dma_start(sint[:], sint_d[:])
            maskt = []
            for o in range(4):
                m = const.tile([128, 512], BF16, tag=f"mask{o}")
                nc.sync.dma_start(m[:], masks_d[o])
                maskt.append(m)

            # persistent per-head tensors
            QT = [persist.tile([128, L], BF16, tag=f"qt{h}", name=f"qt{h}") for h in range(QH)]
            KT = [persist.tile([128, L], BF16, tag=f"kt{g}", name=f"kt{g}") for g in range(KVH)]
            # V extended with a ones column per k-tile: [128, 16*129]
            VE = [persist.tile([128, LT * 129], BF16, tag=f"ve{g}", name=f"ve{g}") for g in range(KVH)]

            def body():
                _phases(nc, tc, ident, cost, sint, maskt, QT, KT, VE,
                        xt_d, wqt_d, wkt_d, wvt_d, wot_d, out_d)

            if n_iters == 1:
                body()
            else:
                with tc.For_i(0, n_iters, 1,
                              hint_engines=(mybir.EngineType.PE,
                                            mybir.EngineType.Activation,
                                            mybir.EngineType.DVE,
                                            mybir.EngineType.SP)):
                    body()
    return nc


def _phases(nc, tc, ident, cost, sint, maskt, QT, KT, VE,
            xt_d, wqt_d, wkt_d, wvt_d, wot_d, out_d):
            # ---------------- phase 1: projections + rope ----------------
            def drain_rope(ps, dst, nq, fold):
                """psum [128,512] f32 -> rope -> dst bf16 [128,512] slice."""
                cols = slice(nq * 512, (nq + 1) * 512)
                raw = rtmp.tile([128, 512], F32, tag="raw")
                nc.scalar.activation(raw[:], ps[:], AF.Copy)
                swap = rtmp.tile([128, 512], F32, tag="swap")
                nc.scalar.activation(swap[0:64, :], ps[64:128, :], AF.Copy)
                nc.scalar.activation(swap[64:128, :], ps[0:64, :], AF.Copy)
                nc.vector.tensor_mul(raw[:], raw[:], cost[:, cols])
                nc.vector.tensor_mul(swap[:], swap[:], sint[:, cols])
                nc.vector.tensor_add(dst[:, cols], raw[:], swap[:])

            with (
                tc.tile_pool(name="xcol", bufs=2) as xcol,
                tc.tile_pool(name="rtmp", bufs=3) as rtmp,
                tc.tile_pool(name="wts", bufs=1) as wts,
                tc.tile_pool(name="pj_psum", bufs=2, space="PSUM") as pj_psum,
            ):
                # single pass over x columns computing Q, K (rope'd, [d, l])
                # and V (direct [l, d] with xT stationary) per 512-wide chunk.
                # DMA dispatch order = SP issue order: wq first (first output
                # rows need only wq), then chunk-0 x columns, then wk/wv.
                wq, wk, wv = [], [], []
                for k in range(KC):
                    w = wts.tile([128, QH * 128], BF16, tag=f"q{k}", name=f"wq{k}")
                    nc.sync.dma_start(w[:], wqt_d[k * 128:(k + 1) * 128, :])
                    wq.append(w)
                xc0 = []
                for k in range(KC):
                    t = xcol.tile([128, 512], BF16, tag=f"x{k}", name=f"xc{k}")
                    nc.sync.dma_start(t[:], xt_d[k * 128:(k + 1) * 128, 0:512])
                    xc0.append(t)
                for k in range(KC):
                    w = wts.tile([128, KVH * 128], BF16, tag=f"k{k}", name=f"wk{k}")
                    nc.sync.dma_start(w[:], wkt_d[k * 128:(k + 1) * 128, :])
                    wk.append(w)
                for k in range(KC):
                    w = wts.tile([128, KVH * 128], BF16, tag=f"v{k}", name=f"wv{k}")
                    nc.sync.dma_start(w[:], wvt_d[k * 128:(k + 1) * 128, :])
                    wv.append(w)

                for nq in range(NQ):
                    if nq == 0:
                        xc = xc0
                    else:
                        xc = []
                        for k in range(KC):
                            t = xcol.tile([128, 512], BF16, tag=f"x{k}", name=f"xc{k}")
                            nc.sync.dma_start(
                                t[:], xt_d[k * 128:(k + 1) * 128, nq * 512:(nq + 1) * 512])
                            xc.append(t)
                    for h in range(QH):
                        ps = pj_psum.tile([128, 512], F32, tag="qk")
                        for k in range(KC):
                            nc.tensor.matmul(
                                ps[:], wq[k][:, h * 128:(h + 1) * 128], xc[k][:],
                                start=(k == 0), stop=(k == KC - 1))
                        drain_rope(ps, QT[h], nq, True)
                    for g in range(KVH):
                        ps = pj_psum.tile([128, 512], F32, tag="qk")
                        for k in range(KC):
                            nc.tensor.matmul(
                                ps[:], wk[k][:, g * 128:(g + 1) * 128], xc[k][:],
                                start=(k == 0), stop=(k == KC - 1))
                        drain_rope(ps, KT[g], nq, False)
                    for sub in range(4):
                        mk = nq * 4 + sub
                        ps = pj_psum.tile([128, KVH * 128], F32, tag="vps")
                        for k in range(KC):
                            nc.tensor.matmul(
                                ps[:], xc[k][:, sub * 128:(sub + 1) * 128], wv[k][:],
                                start=(k == 0), stop=(k == KC - 1))
                        for g in range(KVH):
                            nc.vector.tensor_copy(
                                VE[g][:, mk * 129:mk * 129 + 128],
                                ps[:, g * 128:(g + 1) * 128])
                            nc.vector.memset(
                                VE[g][:, mk * 129 + 128:mk * 129 + 129], 1.0)

            # ---------------- phase 2: attention + o_proj ----------------
            with (
                tc.tile_pool(name="wo", bufs=1) as wop,
                tc.tile_pool(name="pt", bufs=2) as ptp,
                tc.tile_pool(name="tanh", bufs=3) as tanhp,
                tc.tile_pool(name="attnt", bufs=2) as attp,
                tc.tile_pool(name="small", bufs=3) as small,
                tc.tile_pool(name="ostage", bufs=2) as ostage,
                tc.tile_pool(name="sc_psum", bufs=2, space="PSUM") as sc_psum,
                tc.tile_pool(name="pv_psum", bufs=2, space="PSUM") as pv_psum,
                tc.tile_pool(name="op_psum", bufs=2, space="PSUM") as op_psum,
                tc.tile_pool(name="atr_psum", bufs=2, space="PSUM") as atr_psum,
            ):
                WO = []
                for h in range(QH):
                    w = wop.tile([128, D], BF16, tag=f"wo{h}")
                    nc.sync.dma_start(w[:], wot_d[h * 128:(h + 1) * 128, :])
                    WO.append(w)

                for nq in range(NQ):
                    attnT = [attp.tile([128, 512], BF16, tag=f"at{h}", name=f"at{h}") for h in range(QH)]
                    for h in range(QH):
                        g = h // 2
                        nkt = 4 * nq + 4
                        pts = []
                        for mk in range(nkt):
                            o = mk - 4 * nq  # >= 0 on diagonal tiles
                            c0 = max(0, o) * 128  # first column with any valid q
                            w = 512 - c0
                            ps_s = sc_psum.tile([128, 512], F32)
                            nc.tensor.matmul(
                                ps_s[:, 0:w], KT[g][:, mk * 128:(mk + 1) * 128],
                                QT[h][:, nq * 512 + c0:(nq + 1) * 512])
                            tt = tanhp.tile([128, 512], F32, tag="tanh")
                            nc.scalar.activation(
                                tt[:, 0:w], ps_s[:, 0:w], AF.Tanh, scale=SCALE / SOFTCAP)
                            pt = ptp.tile([128, 512], BF16, tag=f"pt{mk}")
                            nc.scalar.activation(
                                pt[:, c0:512], tt[:, 0:w], AF.Exp, scale=SOFTCAP)
                            if o >= 0:
                                nc.vector.tensor_mul(
                                    pt[:, c0:512], pt[:, c0:512], maskt[o][:, c0:512])
                            pts.append(pt)
                        for s in range(4):
                            nks = 4 * nq + s + 1
                            pv = pv_psum.tile([128, 129], F32)
                            for mk in range(nks):
                                nc.tensor.matmul(
                                    pv[:], pts[mk][:, s * 128:(s + 1) * 128],
                                    VE[g][:, mk * 129:(mk + 1) * 129],
                                    start=(mk == 0), stop=(mk == nks - 1))
                            recip = small.tile([128, 1], F32, tag="recip")
                            nc.vector.reciprocal(recip[:], pv[:, 128:129])
                            attn_q = small.tile([128, 128], BF16, tag="attnq")
                            nc.vector.tensor_scalar_mul(attn_q[:], pv[:, 0:128], recip[:])
                            tp = atr_psum.tile([128, 128], BF16, tag="atr")
                            nc.tensor.transpose(tp[:], attn_q[:], ident[:])
                            nc.vector.tensor_copy(attnT[h][:, s * 128:(s + 1) * 128], tp[:])

                    for s in range(4):
                        row = nq * 512 + s * 128
                        ob = ostage.tile([128, D], F32, tag="ob")
                        for j in range(DOUT_CHUNKS):
                            po = op_psum.tile([128, 512], F32)
                            for h in range(QH):
                                nc.tensor.matmul(
                                    po[:], attnT[h][:, s * 128:(s + 1) * 128],
                                    WO[h][:, j * 512:(j + 1) * 512],
                                    start=(h == 0), stop=(h == QH - 1))
                            nc.vector.tensor_copy(ob[:, j * 512:(j + 1) * 512], po[:])
                        nc.sync.dma_start(out_d[row:row + 128, :], ob[:])


_CACHED_NC = {}


def build(n_iters=1):
    if n_iters not in _CACHED_NC:
        nc = bacc.Bacc("TRN2", target_bir_lowering=False, debug=False)
        _emit(nc, n_iters)
        nc.compile()
        _CACHED_NC[n_iters] = nc
    return _CACHED_NC[n_iters]


def host_tables():
    inv_freq = 1.0 / (ROPE_THETA ** (np.arange(0, HEAD_DIM, 2, dtype=np.float32) / HEAD_DIM))
    ang = np.arange(L, dtype=np.float32)[:, None] * inv_freq[None, :]  # [L, 64]
    cos, sin = np.cos(ang), np.sin(ang)
    cosT = np.concatenate([cos.T, cos.T], axis=0).astype(BF16_NP)
    sinT = np.concatenate([-sin.T, sin.T], axis=0).astype(BF16_NP)
    return np.ascontiguousarray(cosT), np.ascontiguousarray(sinT)


def host_masks():
    k = np.arange(128)[:, None]
    q = np.arange(512)[None, :]
    m = np.stack([(q >= k + 128 * o) for o in range(4)]).astype(BF16_NP)
    return np.ascontiguousarray(m)


def make_in_maps(x, wq, wk, wv, wo):
    cosT, sinT = host_tables()
    masks = host_masks()
    xt = np.ascontiguousarray(x.reshape(L, D).T).astype(BF16_NP)
    in_maps = []
    for c in range(N_CORES):
        qs = slice(c * QH * 128, (c + 1) * QH * 128)
        kvs = slice(c * KVH * 128, (c + 1) * KVH * 128)
        in_maps.append({
            "xt": xt,
            "wqt": np.ascontiguousarray(wq[qs].T.astype(BF16_NP)),
            "wkt": np.ascontiguousarray(wk[kvs].T.astype(BF16_NP)),
            "wvt": np.ascontiguousarray(wv[kvs].T.astype(BF16_NP)),
            "wot": np.ascontiguousarray(wo[:, qs].T.astype(BF16_NP)),
            "cost": cosT,
            "sint": sinT,
            "masks": masks,
        })
    return in_maps


def run(inputs, trace=False, trace_kwargs=None):
    from concourse.bass_utils import run_bass_kernel_spmd

    nc = build()
    x = np.asarray(inputs["x"], dtype=np.float32)
    in_maps = make_in_maps(
        x,
        np.asarray(inputs["wq"], dtype=np.float32),
        np.asarray(inputs["wk"], dtype=np.float32),
        np.asarray(inputs["wv"], dtype=np.float32),
        np.asarray(inputs["wo"], dtype=np.float32),
    )
    res = run_bass_kernel_spmd(
        nc, in_maps, core_ids=list(range(N_CORES)),
        trace=trace, **(trace_kwargs or {}))
    out = np.zeros((L, D), dtype=np.float32)
    for c in range(N_CORES):
        out += res.results[c]["out"]
    return out.reshape(x.shape), res


def kernel(**inputs) -> np.ndarray:
    out, _ = run(inputs, trace=False)
    return out

